# revision 1
# baseline (speedup 1.0000x reference)
"""Trainium2 Bass kernel for nn_CombinedLoss (regression MSE + masked binary focal loss).

Data-parallel over 8 NeuronCores: each core reduces its batch shard to
per-class partial sums; the final (tiny) weighted combination happens on host
in float64.

Math (per element of the 13 presence classes, t in {-1, 0, 1}):
    a  = 1 - 2t                (exact in bf16: a in {3, 1, -1})
    z  = x * a                 (z = x for t=0, -x for t=1, 3x garbage for t=-1)
    focal(x, t) = softplus(z) * sigmoid(z)^2      for valid t (t != -1)
    weighted by w_c if t==1 else (1-w_c), masked out for t==-1.

On-device we avoid masking entirely by reducing three per-class sums over ALL
elements (including t==-1 garbage, which is finite) in the *a*-moment basis
(a is already bf16 on chip, so no bf16 copy of t is ever needed):
    S0_c = sum f        Sa_c = sum f*a        Saa_c = sum f*a^2
Host side (per class):   F-1 = (Saa-S0)/8   (t==-1 garbage sum)
    F1 = F-1 - (Sa-S0)/2   (t==1 sum),   F0 = S0 - F-1 - F1   (t==0 sum)
    focal_total = sum_c (1-w_c)*F0_c + w_c*F1_c

softplus/sigmoid use only the `natural_log_exp_and_others` ACT table set
(softplus has no HW table on this toolchain):
    e  = exp(z)            [ACT Exp]
    sp = ln(e + 1)         [ACT Ln,  free bias=+1]  == softplus(z), bf16
    s2 = exp(2*(z - sp))   [ACT Exp, scale=+2]      == sigmoid(z)^2, bf16
so the ACT engine never switches table sets (one ACT_TABLE_LOAD total).

Steady state is DMA-bound (~94us/core HBM roofline at 358 GB/s for the
mandatory 32 MiB read). Per 2-tile group (g=2) the engine budget is
DMA ~5.9us, DVE ~4.9us (a, v, f, fa, faa, q - all bf16 2x-packed except a),
ACT ~5.0us (e, sp, s2), GPSIMD ~4.7us (z, d), PE ~2.3us (14 matmuls).
"""

import sys

if "/opt/trn_rl_repo" not in sys.path:
    sys.path.insert(0, "/opt/trn_rl_repo")

import numpy as np

NCORES = 8
B = 2_097_152
BS = B // NCORES          # 262144 rows per core
P = 128                   # SBUF partitions
RPP = BS // P             # 2048 rows per partition
T = 64                    # rows per tile
NT = RPP // T             # 32 tiles
FD_FULL = T * 16          # 1024 fp32 per partition per tile (4KB DMA run)
FD_C = T * 13             # 832 class elements per partition per tile
FD_R = T * 3              # 192 regression elements per partition per tile
NPART = 3 * FD_C + FD_R   # 2688 partial-sum cells per core


def build(reps: int = 1, g: int = 2, bufs_io: int = 6, bufs_deep: int = 6,
          bufs_mid: int = 5, bufs_low: int = 3, pipe: int = 1):
    import concourse.bacc as bacc
    import concourse.mybir as mybir
    import concourse.tile as tile
    import bass_rust as _bass_rust
    from concourse.hw_specs import get_activation_tables

    dt = mybir.dt
    AF = mybir.ActivationFunctionType
    OP = mybir.AluOpType

    class _Bacc(bacc.Bacc):
        """Pin every activation to the natural_log_exp_and_others table set.

        The default chooser scans act_func_sets in order and picks the first
        set containing each function, so Exp lands in exp_and_others and Ln
        in natural_log — alternating ACT_TABLE_LOADs (~1.3us each) every
        group. All functions this kernel uses (Exp, Ln, Copy) live together
        in natural_log_exp_and_others; blanking the other sets (positions
        preserved, since act_func_set_id is the list index) yields exactly
        one table load for the whole kernel.
        """

        def insert_act_table_loads(self):
            has_activation = any(
                isinstance(i, mybir.InstActivation)
                for b in self.main_func.blocks
                for i in b.instructions
            )
            if not has_activation:
                return
            keep = "natural_log_exp_and_others"
            tables = [
                (name, funcs if name == keep else set())
                for name, funcs in get_activation_tables(self.m.arch).items()
            ]
            _bass_rust.insert_act_table_loads(self, tables)

    G_ = g
    NGROUP_ = NT // G_
    FDGF = G_ * FD_FULL
    FDGC = G_ * FD_C
    FDGR = G_ * FD_R

    nc = _Bacc("TRN2", target_bir_lowering=False, debug=False,
               num_devices=NCORES)
    x_d = nc.dram_tensor("output", [BS, 16], dt.float32, kind="ExternalInput")
    t_d = nc.dram_tensor("target", [BS, 16], dt.float32, kind="ExternalInput")
    po_d = nc.dram_tensor("partials", [1, NPART], dt.float32,
                          kind="ExternalOutput")

    # [128, 32768] per-partition contiguous row blocks
    xv = x_d.ap().rearrange("(p r) c -> p (r c)", p=P)
    tv = t_d.ap().rearrange("(p r) c -> p (r c)", p=P)

    with tile.TileContext(nc) as tc:
        with (
            tc.tile_pool(name="io", bufs=bufs_io) as io_pool,
            tc.tile_pool(name="p8", bufs=bufs_deep) as deep_pool,
            tc.tile_pool(name="p5", bufs=bufs_mid) as mid_pool,
            tc.tile_pool(name="p3", bufs=bufs_low) as low_pool,
            tc.tile_pool(name="p2", bufs=2) as min_pool,
            tc.tile_pool(name="cst", bufs=1) as cst_pool,
            tc.tile_pool(name="acc", bufs=1, space="PSUM") as psum_pool,
        ):
            ones = cst_pool.tile([P, 1], dt.bfloat16, tag="ones")
            nc.vector.memset(ones[:], 1.0)

            p0 = psum_pool.tile([1, FD_C], dt.float32, tag="p0")
            p1 = psum_pool.tile([1, FD_C], dt.float32, tag="p1")
            p2 = psum_pool.tile([1, FD_C], dt.float32, tag="p2")
            pq = psum_pool.tile([1, FD_R], dt.float32, tag="pq")

            # Fully-retimed software pipeline with bf16 cast-DMAs
            # (SWDGE on the gpsimd queue casts fp32->bf16 in the DMA
            # datapath, so every on-chip operand is bf16 and packs 2x/4x on
            # DVE). In steady-state body b every op consumes only values
            # produced in earlier bodies or earlier on its own in-order
            # queue, so no engine stalls:
            #   body b : cast-DMA xg/tg for group b [Pool/SWDGE issue]
            #   b-1    : a = 1-2t, z = x*a [DVE]; d = xr-tr, q = d*d [Pool]
            #   b-2    : e = exp(z), sp = ln(e+1) [ACT, in-order pair]
            #   b-3    : v = z-sp [DVE]
            #   b-4    : s2 = exp(2v) [ACT, 3rd slot]
            #   b-5    : f = sp*s2, fa = f*a, faa = fa*a [DVE];
            #            7 accumulating matmuls [PE]
            for rep in range(reps):
                IO, A, Z, D, Q, SPT, V, S2 = ({} for _ in range(8))
                for b in range(NGROUP_ + 5):
                    g1, g2, g3, g4, g5 = b - 1, b - 2, b - 3, b - 4, b - 5

                    if b < NGROUP_:
                        xg = io_pool.tile([P, FDGF], dt.bfloat16, tag="xg")
                        tg = io_pool.tile([P, FDGF], dt.bfloat16, tag="tg")
                        nc.gpsimd.dma_start(xg[:],
                                            xv[:, b * FDGF:(b + 1) * FDGF])
                        nc.gpsimd.dma_start(tg[:],
                                            tv[:, b * FDGF:(b + 1) * FDGF])
                        IO[b] = (xg, tg)

                    if 0 <= g1 < NGROUP_:
                        xg, tg = IO[g1]
                        x3 = xg[:].rearrange("p (r c) -> p r c", c=16)
                        t3 = tg[:].rearrange("p (r c) -> p r c", c=16)
                        # a = 1 - 2t in bf16 (exact: {3, 1, -1}), 4x packed
                        ag = deep_pool.tile([P, FDGC], dt.bfloat16, tag="a")
                        a3 = ag[:].rearrange("p (r c) -> p r c", c=13)
                        nc.vector.tensor_scalar(
                            a3, t3[:, :, 3:16], -2.0, 1.0, OP.mult, OP.add)
                        A[g1] = ag
                        # z = x * a on DVE (all-bf16, 2x packed)
                        zg = mid_pool.tile([P, FDGC], dt.bfloat16, tag="z")
                        z3 = zg[:].rearrange("p (r c) -> p r c", c=13)
                        nc.vector.tensor_tensor(z3, x3[:, :, 3:16], a3,
                                                OP.mult)
                        Z[g1] = zg
                        # regression on the (otherwise free) Pool engine
                        dg = low_pool.tile([P, FDGR], dt.bfloat16, tag="d")
                        d3 = dg[:].rearrange("p (r c) -> p r c", c=3)
                        nc.gpsimd.tensor_tensor(d3, x3[:, :, 0:3],
                                                t3[:, :, 0:3], OP.subtract)
                        D[g1] = dg
                        qg = deep_pool.tile([P, FDGR], dt.bfloat16, tag="q")
                        nc.gpsimd.tensor_tensor(qg[:], dg[:], dg[:], OP.mult)
                        Q[g1] = qg

                    if 0 <= g2 < NGROUP_:
                        eg = low_pool.tile([P, FDGC], dt.bfloat16, tag="e")
                        nc.scalar.activation(eg[:], Z[g2][:], AF.Exp)
                        spg = mid_pool.tile([P, FDGC], dt.bfloat16, tag="sp")
                        nc.scalar.activation(spg[:], eg[:], AF.Ln,
                                             bias=1.0)
                        SPT[g2] = spg

                    if 0 <= g3 < NGROUP_:
                        vg = low_pool.tile([P, FDGC], dt.bfloat16, tag="v")
                        nc.vector.tensor_tensor(vg[:], Z[g3][:], SPT[g3][:],
                                                OP.subtract)
                        V[g3] = vg

                    if 0 <= g4 < NGROUP_:
                        s2g = low_pool.tile([P, FDGC], dt.bfloat16, tag="s2")
                        nc.scalar.activation(s2g[:], V[g4][:], AF.Exp,
                                             scale=2.0)
                        S2[g4] = s2g

                    if 0 <= g5 < NGROUP_:
                        ag, spg, s2g, qg = A[g5], SPT[g5], S2[g5], Q[g5]
                        fg = min_pool.tile([P, FDGC], dt.bfloat16, tag="f")
                        nc.vector.tensor_tensor(fg[:], spg[:], s2g[:],
                                                OP.mult)
                        fag = min_pool.tile([P, FDGC], dt.bfloat16, tag="fa")
                        nc.vector.tensor_tensor(fag[:], fg[:], ag[:],
                                                OP.mult)
                        faag = min_pool.tile([P, FDGC], dt.bfloat16,
                                             tag="faa")
                        nc.vector.tensor_tensor(faag[:], fag[:], ag[:],
                                                OP.mult)

                        for i in range(G_):
                            j = g5 * G_ + i
                            st = j == 0
                            fin = j == NT - 1
                            off = i * FD_C
                            for (acc, src) in ((p0, fg), (p1, fag),
                                               (p2, faag)):
                                nc.tensor.matmul(acc[:, 0:512], ones[:],
                                                 src[:, off:off + 512],
                                                 start=st, stop=fin)
                                nc.tensor.matmul(acc[:, 512:FD_C], ones[:],
                                                 src[:, off + 512:off + FD_C],
                                                 start=st, stop=fin)
                            nc.tensor.matmul(pq[:], ones[:],
                                             qg[:, i * FD_R:(i + 1) * FD_R],
                                             start=st, stop=fin)

            outt = cst_pool.tile([1, NPART], dt.float32, tag="out")
            nc.scalar.copy(outt[:, 0:FD_C], p0[:])
            nc.scalar.copy(outt[:, FD_C:2 * FD_C], p1[:])
            nc.scalar.copy(outt[:, 2 * FD_C:3 * FD_C], p2[:])
            nc.scalar.copy(outt[:, 3 * FD_C:NPART], pq[:])
            nc.sync.dma_start(po_d.ap(), outt[:])

    nc.compile()
    return nc


# ---------------------------------------------------------------------------
# Cached PJRT executor (jit once per process; later calls are cheap).
# Mirrors concourse.bass2jax.run_bass_via_pjrt for the 8-core SPMD case.
# ---------------------------------------------------------------------------

_EXEC = None


def _get_executor():
    global _EXEC
    if _EXEC is not None:
        return _EXEC

    import jax
    import concourse.mybir as mybir
    from concourse import bass2jax
    from jax.sharding import Mesh, PartitionSpec
    from jax.experimental.shard_map import shard_map

    nc = build(1)
    bass2jax.install_neuronx_cc_hook()

    partition_name = (nc.partition_id_tensor.name
                      if nc.partition_id_tensor else None)
    in_names, out_names, out_avals = [], [], []
    for alloc in nc.m.functions[0].allocations:
        if not isinstance(alloc, mybir.MemoryLocationSet):
            continue
        name = alloc.memorylocations[0].name
        if alloc.kind == "ExternalInput":
            if name != partition_name:
                in_names.append(name)
        elif alloc.kind == "ExternalOutput":
            out_names.append(name)
            out_avals.append(jax.core.ShapedArray(
                tuple(alloc.tensor_shape), mybir.dt.np(alloc.dtype)))

    n_params = len(in_names)
    n_outs = len(out_avals)
    all_in_names = list(in_names) + list(out_names)
    if partition_name is not None:
        all_in_names.append(partition_name)

    def _body(*args):
        operands = list(args)
        if partition_name is not None:
            operands.append(bass2jax.partition_id_tensor())
        return tuple(bass2jax._bass_exec_p.bind(
            *operands,
            out_avals=tuple(out_avals),
            in_names=tuple(all_in_names),
            out_names=tuple(out_names),
            lowering_input_output_aliases=(),
            sim_require_finite=True,
            sim_require_nnan=True,
            nc=nc,
        ))

    devices = jax.devices()[:NCORES]
    mesh = Mesh(np.asarray(devices), ("core",))
    in_specs = (PartitionSpec("core"),) * (n_params + n_outs)
    out_specs = (PartitionSpec("core"),) * n_outs
    donate = tuple(range(n_params, n_params + n_outs))
    sharded = jax.jit(
        shard_map(_body, mesh=mesh, in_specs=in_specs, out_specs=out_specs,
                  check_rep=False),
        donate_argnums=donate, keep_unused=True)

    _EXEC = (sharded, in_names, out_names, out_avals)
    return _EXEC


def run_device_partials(output: np.ndarray, target: np.ndarray) -> np.ndarray:
    """Run the SPMD kernel; returns per-core partials [NCORES, NPART] fp32."""
    sharded, in_names, out_names, out_avals = _get_executor()
    feeds = {"output": np.ascontiguousarray(output, dtype=np.float32),
             "target": np.ascontiguousarray(target, dtype=np.float32)}
    ins = [feeds[n] for n in in_names]
    zeros = [np.zeros((NCORES * a.shape[0],) + a.shape[1:], a.dtype)
             for a in out_avals]
    outs = sharded(*ins, *zeros)
    idx = out_names.index("partials")
    return np.asarray(outs[idx]).reshape(NCORES, NPART)


def combine_partials(partials: np.ndarray,
                     binary_class_weights: np.ndarray) -> np.float32:
    """Host-side fp64 combination of per-core partial sums into the loss.

    Streams are in the a-moment basis (a = 1-2t in {3,1,-1}):
        S0 = F-1 + F0 + F1;  Sa = 3F-1 + F0 - F1;  Saa = 9F-1 + F0 + F1
    """
    p = partials.astype(np.float64).sum(axis=0)
    S0 = p[0:FD_C].reshape(T, 13).sum(axis=0)
    Sa = p[FD_C:2 * FD_C].reshape(T, 13).sum(axis=0)
    Saa = p[2 * FD_C:3 * FD_C].reshape(T, 13).sum(axis=0)
    Q = p[3 * FD_C:NPART].reshape(T, 3).sum(axis=0)
    w = np.asarray(binary_class_weights, dtype=np.float64)
    Fm1 = (Saa - S0) / 8.0
    F1 = Fm1 - (Sa - S0) / 2.0
    F0 = S0 - Fm1 - F1
    focal = np.sum((1.0 - w) * F0 + w * F1)
    mse = Q / float(B)
    loss = 10.0 * mse[0] + mse[1] + mse[2] + focal
    return np.float32(loss)


def kernel(output: np.ndarray, target: np.ndarray,
           binary_class_weights: np.ndarray) -> np.ndarray:
    partials = run_device_partials(output, target)
    return np.asarray(combine_partials(partials, binary_class_weights))

